# revision 70
# baseline (speedup 1.0000x reference)
"""Trainium2 Bass kernel for Attention3D (B=2, N=1024, C=768, H=12, HID=64).

Sharding: 8 cores = 2 batches x 4 query-slices of 256 rows.

Design (transposed attention, bias fused into scores via PSUM accumulation):
  - x is token-rotated per core so its i-slice is at columns 0:256 (no xTq).
  - kT [feat, tok], v (natural, with a ones-column per head for softmax
    denominators) and qT computed on PE.
  - hidden h2[(par,d), j] = relu(A[i,d] - P[d,j]) per i-pair via DVE
    tensor_scalar (add+max, 4x mode).
  - scores^T [j, i] per head and rel-pos bias accumulate into the SAME psum
    bank: bias matmuls use h2 as stationary weights (M=128 tokens j) and a
    packed w2 [128, 24] as the 24-column moving operand; scores matmuls
    (lhsT = kT chunk, rhs = qT chunk) then accumulate on top.  Layout within
    a bank: col = il*12 + h for 32 i's x 12 heads.
  - exp on ACT (no max subtraction; logits are O(1)) -> expST staging bf16.
  - AV: lhsT = expST [j, i-chunk], rhs = [v_h | 1] -> O[i, d] + row sums in
    psum; normalize by reciprocal sums during psum->sbuf copy.
  - out projection from PE-transposed O, direct accumulation over 6 c-chunks.
Host: input marshalling (transpose/rotate/scale/pack) + concat + proj_b add.
"""

import os
import sys

for _p in ("/opt/trn_rl_repo",):
    if _p not in sys.path:
        sys.path.insert(0, _p)

import numpy as np
import ml_dtypes

from contextlib import ExitStack

import concourse.bass as bass
import concourse.bacc as bacc
import concourse.mybir as mybir
import concourse.tile as tile
from concourse import bass_utils
from concourse.masks import make_identity
from bass_rust import add_dep_helper as _add_dep


def _dep(a, b):
    """a must execute after b (same-engine ordering, no semaphore)."""
    _add_dep(a.ins, b.ins, sync=False, reason="psum-accum-order")

BF16 = mybir.dt.bfloat16
F32 = mybir.dt.float32
ALU = mybir.AluOpType
ACTF = mybir.ActivationFunctionType

B, N, C, H, HID = 2, 1024, 768, 12, 64
HD = C // H  # 64
NSLICE = 4
I_LEN = N // NSLICE  # 256
P = 128

LAST_EXEC_NS = None
LAST_RESULTS = None

_CACHE = {}


def _build_program():
    nc = bacc.Bacc(
        "TRN2",
        target_bir_lowering=False,
        debug=False,
        enable_asserts=False,
        num_devices=8,
    )

    # DRAM I/O (per-core inputs; same names for all cores)
    xT = nc.dram_tensor("xT", [C, N], BF16, kind="ExternalInput").ap()
    qwT = nc.dram_tensor("qwT", [C, C], BF16, kind="ExternalInput").ap()
    kwT = nc.dram_tensor("kwT", [C, C], BF16, kind="ExternalInput").ap()
    vwT = nc.dram_tensor("vwT", [C, C], BF16, kind="ExternalInput").ap()
    pwT = nc.dram_tensor("pwT", [C, C], BF16, kind="ExternalInput").ap()
    ptn2 = nc.dram_tensor("ptn2", [P, N], BF16, kind="ExternalInput").ap()
    at2 = nc.dram_tensor("at2", [P, I_LEN // 2], F32, kind="ExternalInput").ap()
    w2pk = nc.dram_tensor("w2pk", [P, 2 * H], BF16, kind="ExternalInput").ap()
    out = nc.dram_tensor("out", [I_LEN, C], F32, kind="ExternalOutput").ap()

    with tile.TileContext(nc) as tc, ExitStack() as ctx:
        consts = ctx.enter_context(tc.tile_pool(name="consts", bufs=1))
        h2p = ctx.enter_context(tc.tile_pool(name="h2p", bufs=5))
        expp = ctx.enter_context(tc.tile_pool(name="expp", bufs=1))
        anp = ctx.enter_context(tc.tile_pool(name="anp", bufs=2))
        outp = ctx.enter_context(tc.tile_pool(name="outp", bufs=2))
        rcp = ctx.enter_context(tc.tile_pool(name="rcp", bufs=2))
        ssp = ctx.enter_context(tc.tile_pool(name="ssp", bufs=4, space="PSUM"))
        avp = ctx.enter_context(tc.tile_pool(name="avp", bufs=1, space="PSUM"))
        mmp = ctx.enter_context(tc.tile_pool(name="mmp", bufs=2, space="PSUM"))

        # ---- staged inputs in SBUF ----
        # DMA queue spreading: h2 deps (at2/ptn2/w2pk) go first on the Pool
        # SWDGE queue (25ns triggers); kwT follows there.  xT+pwT ride the SP
        # HW queue, qwT+vwT the ACT HW queue (ACT is idle early).  This keeps
        # DVE's queue free for compute and gets h2 running within ~1us.
        ptn2_sb = consts.tile([P, N], BF16)
        at2_sb = consts.tile([P, I_LEN // 2], F32)
        w2pk_sb = consts.tile([P, 2 * H], BF16)
        nc.sync.dma_start(at2_sb[:, 0:64], at2[:, 0:64])
        nc.scalar.dma_start(at2_sb[:, 64:128], at2[:, 64:128])
        nc.sync.dma_start(ptn2_sb[:, 0:512], ptn2[:, 0:512])
        nc.scalar.dma_start(ptn2_sb[:, 512:1024], ptn2[:, 512:1024])
        nc.gpsimd.dma_start(w2pk_sb[:], w2pk)
        xT_sb = consts.tile([P, 6, N], BF16)
        qwT_sb = consts.tile([P, 6, C], BF16)
        kwT_sb = consts.tile([P, 6, C], BF16)
        vwT_sb = consts.tile([P, 6, C], BF16)
        pwT_sb = consts.tile([P, 6, C], BF16)
        # kwT first on the SP HW queue (gates the kT matmuls), xT split over
        # SP+ACT, qwT/vwT behind xT on ACT, pwT late on the Pool queue.
        for cc in range(6):
            nc.sync.dma_start(
                kwT_sb[:, cc, :],
                kwT.rearrange("(c p) f -> p c f", p=P)[:, cc, :],
            )
        for jh in range(2):
            for cc in range(6):
                eng = nc.sync if cc % 2 == 0 else nc.scalar
                eng.dma_start(
                    xT_sb[:, cc, jh * 512:(jh + 1) * 512],
                    xT.rearrange("(c p) n -> p c n", p=P)[
                        :, cc, jh * 512:(jh + 1) * 512
                    ],
                )
        for w_sb, w_dram, eng in ((qwT_sb, qwT, nc.scalar),
                                  (vwT_sb, vwT, nc.sync),
                                  (pwT_sb, pwT, nc.gpsimd)):
            for cc in range(6):
                eng.dma_start(
                    w_sb[:, cc, :],
                    w_dram.rearrange("(c p) f -> p c f", p=P)[:, cc, :],
                )
        ident = consts.tile([P, P], BF16)
        make_identity(nc, ident[:])

        kT_sb = consts.tile([P, 6, N], BF16)
        va_sb = consts.tile([P, 8, H * 65], BF16)
        # q^T zero-padded per head parity: qTz[:, 0] has odd-head feature rows
        # zeroed, qTz[:, 1] even-head rows zeroed.  Keeps the scores matmuls at
        # K=128 (tile config (0,0,128,128)) — mixing K=64 quadrant matmuls into
        # an open psum accumulation group wedges the hardware.
        qTz_sb = consts.tile([P, 2, 6, I_LEN], BF16)
        aT_sb = consts.tile([P, 6, I_LEN], BF16)
        # ---- h2 production (DVE): relu(A_col + ptn2), 8 pairs per tile ----
        h2t = []
        for t in range(16):
            ht = h2p.tile([P, 8, N], BF16, tag="h2")
            for s in range(8):
                pg = t * 8 + s
                # last 7 pairs of each 32-pair sg group go to the otherwise
                # idle Pool engine so DVE stops pacing phase 3
                eng = nc.gpsimd if (pg % 32) >= 28 else nc.vector
                eng.tensor_scalar(
                    ht[:, s, :], ptn2_sb[:], at2_sb[:, pg:pg + 1], 0.0,
                    ALU.add, ALU.max,
                )
            h2t.append(ht)

        nc.gpsimd.memset(qTz_sb[64:128, 0, :, :], 0.0)
        nc.gpsimd.memset(qTz_sb[0:64, 1, :, :], 0.0)
        # ones columns of va (col h*65+64 per token-chunk)
        nc.gpsimd.memset(
            va_sb[:].rearrange("p t (h e) -> p t h e", h=H, e=65)[:, :, :, 64:65],
            1.0,
        )

        # ---- qkv projections ----
        # k^T [768 feat, 1024 tok]
        for fc in range(6):
            for jh in range(2):
                ps = mmp.tile([P, 512], F32, tag="mm")
                for cc in range(6):
                    nc.tensor.matmul(
                        ps[:],
                        kwT_sb[:, cc, fc * P:(fc + 1) * P],
                        xT_sb[:, cc, jh * 512:(jh + 1) * 512],
                        start=(cc == 0),
                        stop=(cc == 5),
                    )
                nc.scalar.copy(kT_sb[:, fc, jh * 512:(jh + 1) * 512], ps[:])
        # q^T for the core's i-slice (tokens 0:256 after rotation)
        for fc in range(6):
            ps = mmp.tile([P, I_LEN], F32, tag="mm")
            for cc in range(6):
                nc.tensor.matmul(
                    ps[:],
                    qwT_sb[:, cc, fc * P:(fc + 1) * P],
                    xT_sb[:, cc, 0:I_LEN],
                    start=(cc == 0),
                    stop=(cc == 5),
                )
            nc.scalar.copy(qTz_sb[0:64, 0, fc, :], ps[0:64, :])
            nc.scalar.copy(qTz_sb[64:128, 1, fc, :], ps[64:128, :])
        # v natural [tok, feat] into 65-column head slots
        for tci in range(8):
            for oh in range(2):
                ps = mmp.tile([P, 384], F32, tag="mm")
                for cc in range(6):
                    nc.tensor.matmul(
                        ps[:],
                        xT_sb[:, cc, tci * P:(tci + 1) * P],
                        vwT_sb[:, cc, oh * 384:(oh + 1) * 384],
                        start=(cc == 0),
                        stop=(cc == 5),
                    )
                dst = va_sb[:, tci, oh * 390:oh * 390 + 390].rearrange(
                    "p (h e) -> p h e", h=6, e=65
                )[:, :, 0:64]
                nc.scalar.copy(dst, ps[:])

        # ---- attention (transposed) ----
        def avcol(h):
            return h * 65 if h < 7 else 512 + (h - 7) * 65

        for ic in range(2):
            expt = expp.tile([P, 8, 12 * 128], BF16, tag="exp")
            for sg in range(2):
                for jc in range(8):
                    for s32 in range(2):
                        SS = ssp.tile([P, 512], F32, tag="ss")
                        # bias matmuls first (starter covers the bank); chain
                        # them so the starter executes first, scores last.
                        starter = None
                        bias_mms = []
                        for ph in range(17):
                            if ph < 16:
                                pg = ic * 64 + sg * 32 + s32 * 16 + ph
                                ht = h2t[pg // 8]
                                lhsT = ht[:, pg % 8, jc * P:(jc + 1) * P]
                                wid = 24
                            else:
                                # dummy block at cols 384..392: CoreSim's
                                # pending-byte check for the strided scores APs
                                # walks bytes up to col 392; keep them
                                # non-pending
                                lhsT = ptn2_sb[:, 0:P]
                                wid = 9
                            mm = nc.tensor.matmul(
                                SS[:, 24 * ph:24 * ph + wid],
                                lhsT,
                                w2pk_sb[:, 0:wid],
                                start=(ph == 0),
                                stop=False,
                                skip_group_check=True,
                            )
                            # only the bank-starter must go first; the rest are
                            # disjoint zero-overwrites in any order
                            if starter is None:
                                starter = mm
                            else:
                                _dep(mm, starter)
                            bias_mms.append(mm)
                        # scores accumulate on top: col = il*12 + h
                        ssb = SS[:, 0:384].rearrange(
                            "p (a b h) -> p a b h", a=2, b=16, h=H
                        )
                        i0 = ic * 128 + sg * 64 + s32 * 32
                        prev = None
                        for h in range(12):
                            fc = h // 2
                            mm = nc.tensor.matmul(
                                ssb[:, :, :, h],
                                kT_sb[:, fc, jc * P:(jc + 1) * P],
                                qTz_sb[:, h % 2, fc, i0:i0 + 32],
                                start=False,
                                stop=(h == 11),
                                skip_group_check=True,
                            )
                            # scores run after every bias matmul; chain scores
                            # so the stop-flagged one executes last
                            if prev is None:
                                for bm in bias_mms:
                                    _dep(mm, bm)
                            else:
                                _dep(mm, prev)
                            prev = mm
                        # exp -> expST staging: col = h*128 + sg*64 + s32*32 + il
                        dst = expt[:, jc, :].rearrange(
                            "p (h g i) -> p g i h", h=H, g=4, i=32
                        )[:, sg * 2 + s32]
                        nc.scalar.activation(
                            dst, SS[:, 0:384], ACTF.Exp, bias=0.0, scale=1.0
                        )
            # AV for the full 128-row i-block (M=128)
            avps = avp.tile([P, 1024], F32, tag="av")
            av_prev = None
            for jc in range(8):
                for h in range(12):
                    hc = avcol(h)
                    mm = nc.tensor.matmul(
                        avps[:, hc:hc + 65],
                        expt[:, jc, h * 128:(h + 1) * 128],
                        va_sb[:, jc, h * 65:h * 65 + 65],
                        start=(jc == 0 and h in (0, 7)),
                        stop=(jc == 7 and h in (6, 11)),
                        skip_group_check=True,
                    )
                    # chain: starters first, stop-flagged matmuls last
                    if av_prev is not None:
                        _dep(mm, av_prev)
                    av_prev = mm

            # normalize + pack O natural [128 i, 768] bf16
            rc = rcp.tile([P, H], F32, tag="rc")
            av_view = avps[:].rearrange("p (b c) -> p b c", b=2, c=512)
            nc.vector.reciprocal(
                rc[:, 0:7],
                av_view[:, 0, 0:455].rearrange(
                    "p (h e) -> p h e", h=7, e=65)[:, :, 64],
            )
            nc.vector.reciprocal(
                rc[:, 7:12],
                av_view[:, 1, 0:325].rearrange(
                    "p (h e) -> p h e", h=5, e=65)[:, :, 64],
            )
            an = anp.tile([P, C], BF16, tag="an")
            for h in range(12):
                hc = avcol(h)
                nc.vector.tensor_scalar_mul(
                    an[:, h * 64:(h + 1) * 64],
                    avps[:, hc:hc + 64],
                    rc[:, h:h + 1],
                )
            # transpose O chunk [128 i, 128 c] -> aT [128 c, 128 i]
            for cc in range(6):
                pst = mmp.tile([P, P], BF16, tag="mm")
                nc.tensor.transpose(
                    pst[:], an[:, cc * P:(cc + 1) * P], ident[:]
                )
                nc.vector.tensor_copy(
                    aT_sb[:, cc, ic * P:(ic + 1) * P], pst[:])
            # out projection
            for oh in range(2):
                ps = mmp.tile([P, 384], F32, tag="mm")
                for cc in range(6):
                    nc.tensor.matmul(
                        ps[:],
                        aT_sb[:, cc, ic * P:(ic + 1) * P],
                        pwT_sb[:, cc, oh * 384:(oh + 1) * 384],
                        start=(cc == 0),
                        stop=(cc == 5),
                    )
                oc = outp.tile([P, 384], F32, tag="oc")
                nc.scalar.copy(oc[:], ps[:])
                # split output DMA over the idle HW queues (SP + ACT) to
                # shrink the end-of-kernel tail
                for q, eng in ((0, nc.sync), (1, nc.scalar)):
                    eng.dma_start(
                        out[ic * P:(ic + 1) * P,
                            oh * 384 + q * 192:oh * 384 + (q + 1) * 192],
                        oc[:, q * 192:(q + 1) * 192],
                    )

    nc.compile()
    return nc


def _prep_inputs(x, coords_3d, qkv_w, proj_w, mlp_w1, mlp_b1, mlp_w2):
    bf = ml_dtypes.bfloat16
    in_maps = []
    qw = (qkv_w[0:C] * (HD ** -0.5)).astype(np.float32)
    kw = qkv_w[C:2 * C]
    vw = qkv_w[2 * C:3 * C]
    qwT = np.ascontiguousarray(qw.T).astype(bf)
    kwT = np.ascontiguousarray(kw.T).astype(bf)
    vwT = np.ascontiguousarray(vw.T).astype(bf)
    pwT = np.ascontiguousarray(proj_w.T).astype(bf)
    # w2pk[par2*64+d, par*12+h] = (par==par2) * w2[h, d]
    w2pk = np.zeros((P, 2 * H), np.float32)
    w2pk[0:HID, 0:H] = mlp_w2.T
    w2pk[HID:2 * HID, H:2 * H] = mlp_w2.T
    w2pk = w2pk.astype(bf)

    for b in range(B):
        cb = coords_3d[b].astype(np.float32)
        mv = cb.max(axis=0) - cb.min(axis=0) + 1e-6
        cn = cb / mv
        Pm = cn @ mlp_w1.T.astype(np.float32)          # (1024, 64)
        Am = Pm + mlp_b1.astype(np.float32)            # (1024, 64)
        nPmT = -Pm.T                                   # (64, 1024)
        xT_b = np.ascontiguousarray(x[b].T).astype(np.float32)  # (768, 1024)
        for s in range(NSLICE):
            i0 = s * I_LEN
            # token rotation: column j' holds token (j' + i0) % N
            xTr = np.roll(xT_b, -i0, axis=1).astype(bf)
            ptn2 = np.empty((P, N), np.float32)
            ptn2[0:HID] = np.roll(nPmT, -i0, axis=1)
            ptn2[HID:2 * HID] = ptn2[0:HID]
            at2 = np.empty((P, I_LEN // 2), np.float32)
            Al = Am[i0:i0 + I_LEN]
            at2[0:HID] = Al[0::2].T
            at2[HID:2 * HID] = Al[1::2].T
            in_maps.append({
                "xT": xTr,
                "qwT": qwT,
                "kwT": kwT,
                "vwT": vwT,
                "pwT": pwT,
                "ptn2": ptn2.astype(bf),
                "at2": at2.astype(np.float32),
                "w2pk": w2pk,
            })
    return in_maps


def kernel(x, coords_3d, qkv_w, proj_w, proj_b, mlp_w1, mlp_b1, mlp_w2, mlp_b2):
    global LAST_EXEC_NS, LAST_RESULTS
    x = np.asarray(x, np.float32)
    coords_3d = np.asarray(coords_3d, np.float32)
    qkv_w = np.asarray(qkv_w, np.float32)
    proj_w = np.asarray(proj_w, np.float32)
    proj_b = np.asarray(proj_b, np.float32)
    mlp_w1 = np.asarray(mlp_w1, np.float32)
    mlp_b1 = np.asarray(mlp_b1, np.float32)
    mlp_w2 = np.asarray(mlp_w2, np.float32)

    if "nc" not in _CACHE:
        _CACHE["nc"] = _build_program()
    nc = _CACHE["nc"]

    in_maps = _prep_inputs(x, coords_3d, qkv_w, proj_w, mlp_w1, mlp_b1, mlp_w2)
    trace = bool(int(os.environ.get("KERNEL_TRACE", "0")))
    res = bass_utils.run_bass_kernel_spmd(
        nc, in_maps, list(range(8)), trace=trace
    )
    LAST_EXEC_NS = res.exec_time_ns
    LAST_RESULTS = res
    full = np.empty((B, N, C), np.float32)
    ci = 0
    for b in range(B):
        for s in range(NSLICE):
            full[b, s * I_LEN:(s + 1) * I_LEN] = res.results[ci]["out"]
            ci += 1
    full += proj_b[None, None, :]
    return full


# revision 71
# speedup vs baseline: 1.0072x; 1.0072x over previous
"""Trainium2 Bass kernel for Attention3D (B=2, N=1024, C=768, H=12, HID=64).

Sharding: 8 cores = 2 batches x 4 query-slices of 256 rows.

Design (transposed attention, bias fused into scores via PSUM accumulation):
  - x is token-rotated per core so its i-slice is at columns 0:256 (no xTq).
  - kT [feat, tok], v (natural, with a ones-column per head for softmax
    denominators) and qT computed on PE.
  - hidden h2[(par,d), j] = relu(A[i,d] - P[d,j]) per i-pair via DVE
    tensor_scalar (add+max, 4x mode).
  - scores^T [j, i] per head and rel-pos bias accumulate into the SAME psum
    bank: bias matmuls use h2 as stationary weights (M=128 tokens j) and a
    packed w2 [128, 24] as the 24-column moving operand; scores matmuls
    (lhsT = kT chunk, rhs = qT chunk) then accumulate on top.  Layout within
    a bank: col = il*12 + h for 32 i's x 12 heads.
  - exp on ACT (no max subtraction; logits are O(1)) -> expST staging bf16.
  - AV: lhsT = expST [j, i-chunk], rhs = [v_h | 1] -> O[i, d] + row sums in
    psum; normalize by reciprocal sums during psum->sbuf copy.
  - out projection from PE-transposed O, direct accumulation over 6 c-chunks.
Host: input marshalling (transpose/rotate/scale/pack) + concat + proj_b add.
"""

import os
import sys

for _p in ("/opt/trn_rl_repo",):
    if _p not in sys.path:
        sys.path.insert(0, _p)

import numpy as np
import ml_dtypes

from contextlib import ExitStack

import concourse.bass as bass
import concourse.bacc as bacc
import concourse.mybir as mybir
import concourse.tile as tile
from concourse import bass_utils
from concourse.masks import make_identity
from bass_rust import add_dep_helper as _add_dep


def _dep(a, b):
    """a must execute after b (same-engine ordering, no semaphore)."""
    _add_dep(a.ins, b.ins, sync=False, reason="psum-accum-order")

BF16 = mybir.dt.bfloat16
F32 = mybir.dt.float32
ALU = mybir.AluOpType
ACTF = mybir.ActivationFunctionType

B, N, C, H, HID = 2, 1024, 768, 12, 64
HD = C // H  # 64
NSLICE = 4
I_LEN = N // NSLICE  # 256
P = 128

LAST_EXEC_NS = None
LAST_RESULTS = None

_CACHE = {}


def _build_program():
    nc = bacc.Bacc(
        "TRN2",
        target_bir_lowering=False,
        debug=False,
        enable_asserts=False,
        num_devices=8,
    )

    # DRAM I/O (per-core inputs; same names for all cores)
    xT = nc.dram_tensor("xT", [C, N], BF16, kind="ExternalInput").ap()
    qwT = nc.dram_tensor("qwT", [C, C], BF16, kind="ExternalInput").ap()
    kwT = nc.dram_tensor("kwT", [C, C], BF16, kind="ExternalInput").ap()
    vwT = nc.dram_tensor("vwT", [C, C], BF16, kind="ExternalInput").ap()
    pwT = nc.dram_tensor("pwT", [C, C], BF16, kind="ExternalInput").ap()
    ptn2 = nc.dram_tensor("ptn2", [P, N], BF16, kind="ExternalInput").ap()
    at2 = nc.dram_tensor("at2", [P, I_LEN // 2], F32, kind="ExternalInput").ap()
    w2pk = nc.dram_tensor("w2pk", [P, 2 * H], BF16, kind="ExternalInput").ap()
    out = nc.dram_tensor("out", [I_LEN, C], F32, kind="ExternalOutput").ap()

    with tile.TileContext(nc) as tc, ExitStack() as ctx:
        consts = ctx.enter_context(tc.tile_pool(name="consts", bufs=1))
        h2p = ctx.enter_context(tc.tile_pool(name="h2p", bufs=5))
        expp = ctx.enter_context(tc.tile_pool(name="expp", bufs=1))
        anp = ctx.enter_context(tc.tile_pool(name="anp", bufs=2))
        outp = ctx.enter_context(tc.tile_pool(name="outp", bufs=2))
        rcp = ctx.enter_context(tc.tile_pool(name="rcp", bufs=2))
        ssp = ctx.enter_context(tc.tile_pool(name="ssp", bufs=4, space="PSUM"))
        avp = ctx.enter_context(tc.tile_pool(name="avp", bufs=1, space="PSUM"))
        mmp = ctx.enter_context(tc.tile_pool(name="mmp", bufs=2, space="PSUM"))

        # ---- staged inputs in SBUF ----
        # DMA queue spreading: h2 deps (at2/ptn2/w2pk) go first on the Pool
        # SWDGE queue (25ns triggers); kwT follows there.  xT+pwT ride the SP
        # HW queue, qwT+vwT the ACT HW queue (ACT is idle early).  This keeps
        # DVE's queue free for compute and gets h2 running within ~1us.
        ptn2_sb = consts.tile([P, N], BF16)
        at2_sb = consts.tile([P, I_LEN // 2], F32)
        w2pk_sb = consts.tile([P, 2 * H], BF16)
        nc.sync.dma_start(at2_sb[:, 0:64], at2[:, 0:64])
        nc.scalar.dma_start(at2_sb[:, 64:128], at2[:, 64:128])
        nc.sync.dma_start(ptn2_sb[:, 0:512], ptn2[:, 0:512])
        nc.scalar.dma_start(ptn2_sb[:, 512:1024], ptn2[:, 512:1024])
        nc.gpsimd.dma_start(w2pk_sb[:], w2pk)
        xT_sb = consts.tile([P, 6, N], BF16)
        qwT_sb = consts.tile([P, 6, C], BF16)
        kwT_sb = consts.tile([P, 6, C], BF16)
        vwT_sb = consts.tile([P, 6, C], BF16)
        pwT_sb = consts.tile([P, 6, C], BF16)
        # kwT first on the SP HW queue (gates the kT matmuls), xT split over
        # SP+ACT, qwT/vwT behind xT on ACT, pwT late on the Pool queue.
        for cc in range(6):
            nc.sync.dma_start(
                kwT_sb[:, cc, :],
                kwT.rearrange("(c p) f -> p c f", p=P)[:, cc, :],
            )
        for jh in range(2):
            for cc in range(6):
                eng = nc.sync if cc % 2 == 0 else nc.scalar
                eng.dma_start(
                    xT_sb[:, cc, jh * 512:(jh + 1) * 512],
                    xT.rearrange("(c p) n -> p c n", p=P)[
                        :, cc, jh * 512:(jh + 1) * 512
                    ],
                )
        for w_sb, w_dram, eng in ((qwT_sb, qwT, nc.scalar),
                                  (vwT_sb, vwT, nc.sync),
                                  (pwT_sb, pwT, nc.gpsimd)):
            for cc in range(6):
                eng.dma_start(
                    w_sb[:, cc, :],
                    w_dram.rearrange("(c p) f -> p c f", p=P)[:, cc, :],
                )
        ident = consts.tile([P, P], BF16)
        make_identity(nc, ident[:])

        kT_sb = consts.tile([P, 6, N], BF16)
        va_sb = consts.tile([P, 8, H * 65], BF16)
        # q^T zero-padded per head parity: qTz[:, 0] has odd-head feature rows
        # zeroed, qTz[:, 1] even-head rows zeroed.  Keeps the scores matmuls at
        # K=128 (tile config (0,0,128,128)) — mixing K=64 quadrant matmuls into
        # an open psum accumulation group wedges the hardware.
        qTz_sb = consts.tile([P, 2, 6, I_LEN], BF16)
        aT_sb = consts.tile([P, 6, I_LEN], BF16)
        # ---- h2 production (DVE): relu(A_col + ptn2), 8 pairs per tile ----
        h2t = []
        for t in range(16):
            ht = h2p.tile([P, 8, N], BF16, tag="h2")
            for s in range(8):
                pg = t * 8 + s
                # last 7 pairs of each 32-pair sg group go to the otherwise
                # idle Pool engine so DVE stops pacing phase 3
                eng = nc.gpsimd if (pg % 32) >= 28 else nc.vector
                eng.tensor_scalar(
                    ht[:, s, :], ptn2_sb[:], at2_sb[:, pg:pg + 1], 0.0,
                    ALU.add, ALU.max,
                )
            h2t.append(ht)

        nc.gpsimd.memset(qTz_sb[64:128, 0, :, :], 0.0)
        nc.gpsimd.memset(qTz_sb[0:64, 1, :, :], 0.0)
        # ones columns of va (col h*65+64 per token-chunk)
        nc.gpsimd.memset(
            va_sb[:].rearrange("p t (h e) -> p t h e", h=H, e=65)[:, :, :, 64:65],
            1.0,
        )

        # ---- qkv projections ----
        # k^T [768 feat, 1024 tok]
        for fc in range(6):
            for jh in range(2):
                ps = mmp.tile([P, 512], F32, tag="mm")
                for cc in range(6):
                    nc.tensor.matmul(
                        ps[:],
                        kwT_sb[:, cc, fc * P:(fc + 1) * P],
                        xT_sb[:, cc, jh * 512:(jh + 1) * 512],
                        start=(cc == 0),
                        stop=(cc == 5),
                    )
                nc.scalar.copy(kT_sb[:, fc, jh * 512:(jh + 1) * 512], ps[:])
        # q^T for the core's i-slice (tokens 0:256 after rotation)
        for fc in range(6):
            ps = mmp.tile([P, I_LEN], F32, tag="mm")
            for cc in range(6):
                nc.tensor.matmul(
                    ps[:],
                    qwT_sb[:, cc, fc * P:(fc + 1) * P],
                    xT_sb[:, cc, 0:I_LEN],
                    start=(cc == 0),
                    stop=(cc == 5),
                )
            nc.scalar.copy(qTz_sb[0:64, 0, fc, :], ps[0:64, :])
            nc.scalar.copy(qTz_sb[64:128, 1, fc, :], ps[64:128, :])
        # v natural [tok, feat] into 65-column head slots
        for tci in range(8):
            for oh in range(2):
                ps = mmp.tile([P, 384], F32, tag="mm")
                for cc in range(6):
                    nc.tensor.matmul(
                        ps[:],
                        xT_sb[:, cc, tci * P:(tci + 1) * P],
                        vwT_sb[:, cc, oh * 384:(oh + 1) * 384],
                        start=(cc == 0),
                        stop=(cc == 5),
                    )
                dst = va_sb[:, tci, oh * 390:oh * 390 + 390].rearrange(
                    "p (h e) -> p h e", h=6, e=65
                )[:, :, 0:64]
                nc.scalar.copy(dst, ps[:])

        # ---- attention (transposed) ----
        def avcol(h):
            return h * 65 if h < 7 else 512 + (h - 7) * 65

        for ic in range(2):
            expt = expp.tile([P, 8, 12 * 128], BF16, tag="exp")
            for sg in range(2):
                for jc in range(8):
                    for s32 in range(2):
                        SS = ssp.tile([P, 512], F32, tag="ss")
                        # bias matmuls first (starter covers the bank); chain
                        # them so the starter executes first, scores last.
                        starter = None
                        bias_mms = []
                        for ph in range(17):
                            if ph < 16:
                                pg = ic * 64 + sg * 32 + s32 * 16 + ph
                                ht = h2t[pg // 8]
                                lhsT = ht[:, pg % 8, jc * P:(jc + 1) * P]
                                wid = 24
                            else:
                                # dummy block at cols 384..392: CoreSim's
                                # pending-byte check for the strided scores APs
                                # walks bytes up to col 392; keep them
                                # non-pending
                                lhsT = ptn2_sb[:, 0:P]
                                wid = 9
                            mm = nc.tensor.matmul(
                                SS[:, 24 * ph:24 * ph + wid],
                                lhsT,
                                w2pk_sb[:, 0:wid],
                                start=(ph == 0),
                                stop=False,
                                skip_group_check=True,
                            )
                            # only the bank-starter must go first; the rest are
                            # disjoint zero-overwrites in any order
                            if starter is None:
                                starter = mm
                            else:
                                _dep(mm, starter)
                            bias_mms.append(mm)
                        # scores accumulate on top: col = il*12 + h
                        ssb = SS[:, 0:384].rearrange(
                            "p (a b h) -> p a b h", a=2, b=16, h=H
                        )
                        i0 = ic * 128 + sg * 64 + s32 * 32
                        prev = None
                        for h in range(12):
                            fc = h // 2
                            mm = nc.tensor.matmul(
                                ssb[:, :, :, h],
                                kT_sb[:, fc, jc * P:(jc + 1) * P],
                                qTz_sb[:, h % 2, fc, i0:i0 + 32],
                                start=False,
                                stop=(h == 11),
                                skip_group_check=True,
                            )
                            # scores run after every bias matmul; chain scores
                            # so the stop-flagged one executes last
                            if prev is None:
                                for bm in bias_mms:
                                    _dep(mm, bm)
                            else:
                                _dep(mm, prev)
                            prev = mm
                        # exp -> expST staging: col = h*128 + sg*64 + s32*32 + il
                        dst = expt[:, jc, :].rearrange(
                            "p (h g i) -> p g i h", h=H, g=4, i=32
                        )[:, sg * 2 + s32]
                        nc.scalar.activation(
                            dst, SS[:, 0:384], ACTF.Exp, bias=0.0, scale=1.0
                        )
            # AV for the full 128-row i-block (M=128)
            avps = avp.tile([P, 1024], F32, tag="av")
            av_prev = None
            for jc in range(8):
                for h in range(12):
                    hc = avcol(h)
                    mm = nc.tensor.matmul(
                        avps[:, hc:hc + 65],
                        expt[:, jc, h * 128:(h + 1) * 128],
                        va_sb[:, jc, h * 65:h * 65 + 65],
                        start=(jc == 0 and h in (0, 7)),
                        stop=(jc == 7 and h in (6, 11)),
                        skip_group_check=True,
                    )
                    # chain: starters first, stop-flagged matmuls last
                    if av_prev is not None:
                        _dep(mm, av_prev)
                    av_prev = mm

            # normalize + pack O natural [128 i, 768] bf16
            rc = rcp.tile([P, H], F32, tag="rc")
            av_view = avps[:].rearrange("p (b c) -> p b c", b=2, c=512)
            nc.vector.reciprocal(
                rc[:, 0:7],
                av_view[:, 0, 0:455].rearrange(
                    "p (h e) -> p h e", h=7, e=65)[:, :, 64],
            )
            nc.vector.reciprocal(
                rc[:, 7:12],
                av_view[:, 1, 0:325].rearrange(
                    "p (h e) -> p h e", h=5, e=65)[:, :, 64],
            )
            an = anp.tile([P, C], BF16, tag="an")
            for h in range(12):
                hc = avcol(h)
                nc.vector.tensor_scalar_mul(
                    an[:, h * 64:(h + 1) * 64],
                    avps[:, hc:hc + 64],
                    rc[:, h:h + 1],
                )
            # transpose O chunk [128 i, 128 c] -> aT [128 c, 128 i]
            for cc in range(6):
                pst = mmp.tile([P, P], BF16, tag="mm")
                nc.tensor.transpose(
                    pst[:], an[:, cc * P:(cc + 1) * P], ident[:]
                )
                nc.vector.tensor_copy(
                    aT_sb[:, cc, ic * P:(ic + 1) * P], pst[:])
            # out projection
            for oh in range(2):
                ps = mmp.tile([P, 384], F32, tag="mm")
                for cc in range(6):
                    nc.tensor.matmul(
                        ps[:],
                        aT_sb[:, cc, ic * P:(ic + 1) * P],
                        pwT_sb[:, cc, oh * 384:(oh + 1) * 384],
                        start=(cc == 0),
                        stop=(cc == 5),
                    )
                oc = outp.tile([P, 384], F32, tag="oc")
                # ic0's staging copy on DVE (idle then) keeps ACT's exp
                # stream for ic1 uninterrupted; ic1's stays on ACT (idle end)
                if ic == 0:
                    nc.vector.tensor_copy(oc[:], ps[:])
                else:
                    nc.scalar.copy(oc[:], ps[:])
                # split output DMA over the idle HW queues (SP + ACT) to
                # shrink the end-of-kernel tail
                for q, eng in ((0, nc.sync), (1, nc.scalar)):
                    eng.dma_start(
                        out[ic * P:(ic + 1) * P,
                            oh * 384 + q * 192:oh * 384 + (q + 1) * 192],
                        oc[:, q * 192:(q + 1) * 192],
                    )

    nc.compile()
    return nc


def _prep_inputs(x, coords_3d, qkv_w, proj_w, mlp_w1, mlp_b1, mlp_w2):
    bf = ml_dtypes.bfloat16
    in_maps = []
    qw = (qkv_w[0:C] * (HD ** -0.5)).astype(np.float32)
    kw = qkv_w[C:2 * C]
    vw = qkv_w[2 * C:3 * C]
    qwT = np.ascontiguousarray(qw.T).astype(bf)
    kwT = np.ascontiguousarray(kw.T).astype(bf)
    vwT = np.ascontiguousarray(vw.T).astype(bf)
    pwT = np.ascontiguousarray(proj_w.T).astype(bf)
    # w2pk[par2*64+d, par*12+h] = (par==par2) * w2[h, d]
    w2pk = np.zeros((P, 2 * H), np.float32)
    w2pk[0:HID, 0:H] = mlp_w2.T
    w2pk[HID:2 * HID, H:2 * H] = mlp_w2.T
    w2pk = w2pk.astype(bf)

    for b in range(B):
        cb = coords_3d[b].astype(np.float32)
        mv = cb.max(axis=0) - cb.min(axis=0) + 1e-6
        cn = cb / mv
        Pm = cn @ mlp_w1.T.astype(np.float32)          # (1024, 64)
        Am = Pm + mlp_b1.astype(np.float32)            # (1024, 64)
        nPmT = -Pm.T                                   # (64, 1024)
        xT_b = np.ascontiguousarray(x[b].T).astype(np.float32)  # (768, 1024)
        for s in range(NSLICE):
            i0 = s * I_LEN
            # token rotation: column j' holds token (j' + i0) % N
            xTr = np.roll(xT_b, -i0, axis=1).astype(bf)
            ptn2 = np.empty((P, N), np.float32)
            ptn2[0:HID] = np.roll(nPmT, -i0, axis=1)
            ptn2[HID:2 * HID] = ptn2[0:HID]
            at2 = np.empty((P, I_LEN // 2), np.float32)
            Al = Am[i0:i0 + I_LEN]
            at2[0:HID] = Al[0::2].T
            at2[HID:2 * HID] = Al[1::2].T
            in_maps.append({
                "xT": xTr,
                "qwT": qwT,
                "kwT": kwT,
                "vwT": vwT,
                "pwT": pwT,
                "ptn2": ptn2.astype(bf),
                "at2": at2.astype(np.float32),
                "w2pk": w2pk,
            })
    return in_maps


def kernel(x, coords_3d, qkv_w, proj_w, proj_b, mlp_w1, mlp_b1, mlp_w2, mlp_b2):
    global LAST_EXEC_NS, LAST_RESULTS
    x = np.asarray(x, np.float32)
    coords_3d = np.asarray(coords_3d, np.float32)
    qkv_w = np.asarray(qkv_w, np.float32)
    proj_w = np.asarray(proj_w, np.float32)
    proj_b = np.asarray(proj_b, np.float32)
    mlp_w1 = np.asarray(mlp_w1, np.float32)
    mlp_b1 = np.asarray(mlp_b1, np.float32)
    mlp_w2 = np.asarray(mlp_w2, np.float32)

    if "nc" not in _CACHE:
        _CACHE["nc"] = _build_program()
    nc = _CACHE["nc"]

    in_maps = _prep_inputs(x, coords_3d, qkv_w, proj_w, mlp_w1, mlp_b1, mlp_w2)
    trace = bool(int(os.environ.get("KERNEL_TRACE", "0")))
    res = bass_utils.run_bass_kernel_spmd(
        nc, in_maps, list(range(8)), trace=trace
    )
    LAST_EXEC_NS = res.exec_time_ns
    LAST_RESULTS = res
    full = np.empty((B, N, C), np.float32)
    ci = 0
    for b in range(B):
        for s in range(NSLICE):
            full[b, s * I_LEN:(s + 1) * I_LEN] = res.results[ci]["out"]
            ci += 1
    full += proj_b[None, None, :]
    return full
